# revision 4
# baseline (speedup 1.0000x reference)
"""MoE gate routing kernel for Trainium2 (8 NeuronCores, token-sharded).

Reference computation (per token):
    logits = x @ weight.T            # [T, 256], fp32
    scores = sigmoid(logits) + bias
    topk_weight, topk_idx = top_k(scores, 8)
    topk_weight = topk_weight / (sum(topk_weight) + 1e-20) * 2.5

Design (per core, 2048 tokens):
  - x arrives natural [2048, 4096]; PE transposes 128x128 blocks (fp32,
    bitwise exact) into [h, t] layout via PSUM, DVE/ACT copy back to SBUF.
  - fp32 matmuls accumulate scores^T in PSUM as [128e, 512t] tiles
    (2 expert halves x 4 token groups, K=4096 over 32 h-tiles).
  - scores^T transposed back to [t, e] via PE, sigmoid fused into the
    PSUM->SBUF copy on ACT, bias added on DVE (bias pre-broadcast from host).
  - vector.max / max_index give the top-8 values + indices per token row
    (hardware semantics match jax.lax.top_k: descending, first-index ties).
  - normalize: w = mv / sum(mv) * 2.5 in one tensor_scalar op.
"""

import sys

sys.path.insert(0, "/opt/trn_rl_repo")

import numpy as np

import concourse.bacc as bacc
import concourse.tile as tile
from concourse import masks, mybir
from concourse.bass_utils import run_bass_kernel_spmd

F32 = mybir.dt.float32
U32 = mybir.dt.uint32

N_CORES = 8
T_LOC = 2048          # tokens per core
H = 4096
E = 256
TOP_K = 8
SCALE = 2.5
KT = H // 128         # 32 h-tiles
TG = T_LOC // 512     # 4 token groups of 512
OUTW = 16             # 8 weights + 8 idx (bits) per token row


def build_nc(reps=1):
    nc = bacc.Bacc()
    x_in = nc.dram_tensor("x", [T_LOC, H], F32, kind="ExternalInput")
    wT_in = nc.dram_tensor("wT", [H, E], F32, kind="ExternalInput")
    bias_in = nc.dram_tensor("biasb", [128, E], F32, kind="ExternalInput")
    out = nc.dram_tensor("out", [T_LOC // 128, 128, OUTW], F32, kind="ExternalOutput")

    with tile.TileContext(nc) as tc:
        with (
            tc.tile_pool(name="consts", bufs=1) as consts,
            tc.tile_pool(name="xnat", bufs=6) as xnat_pool,
            tc.tile_pool(name="xT", bufs=3) as xT_pool,
            tc.tile_pool(name="scT", bufs=2) as scT_pool,
            tc.tile_pool(name="sc", bufs=2) as sc_pool,
            tc.tile_pool(name="small", bufs=2) as small,
            tc.tile_pool(name="outp", bufs=2) as outp,
            tc.tile_pool(name="psx", bufs=2, space="PSUM") as psx_pool,
            tc.tile_pool(name="pssc", bufs=2, space="PSUM") as pssc_pool,
            tc.tile_pool(name="pss", bufs=2, space="PSUM") as pss_pool,
        ):
            ident = consts.tile([128, 128], F32)
            masks.make_identity(nc, ident[:])

            # weights (pre-transposed on host): [H, E] -> [128, KT*E]
            wT_sb = consts.tile([128, KT * E], F32)
            nc.sync.dma_start(
                wT_sb[:].rearrange("p (k e) -> p k e", k=KT),
                wT_in[:, :].rearrange("(k p) e -> p k e", p=128),
            )
            biasb = consts.tile([128, E], F32)
            nc.sync.dma_start(biasb[:], bias_in[:, :])

            for tg in [t for _ in range(reps) for t in range(TG)]:
                # ---- load 4 x-row tiles [128 tokens, 4096] ----
                xts = []
                for tt in range(4):
                    xa = xnat_pool.tile([128, H], F32, tag="xnat")
                    nc.sync.dma_start(xa[:], x_in[(tg * 4 + tt) * 128:(tg * 4 + tt + 1) * 128, :])
                    xts.append(xa)

                ps_sc0 = pssc_pool.tile([128, 512], F32, tag="pssc")
                ps_sc1 = pssc_pool.tile([128, 512], F32, tag="pssc")

                for h in range(KT):
                    psx = psx_pool.tile([128, 512], F32, tag="psx")
                    for tt in range(4):
                        nc.tensor.transpose(
                            psx[:, tt * 128:(tt + 1) * 128],
                            xts[tt][:, h * 128:(h + 1) * 128],
                            ident[:],
                        )
                    xT_h = xT_pool.tile([128, 512], F32, tag="xT")
                    if h % 2 == 0:
                        nc.scalar.activation(xT_h[:], psx[:], mybir.ActivationFunctionType.Copy)
                    else:
                        nc.vector.tensor_copy(xT_h[:], psx[:])
                    nc.tensor.matmul(
                        ps_sc0[:], wT_sb[:, h * E:h * E + 128], xT_h[:],
                        start=(h == 0), stop=(h == KT - 1),
                    )
                    nc.tensor.matmul(
                        ps_sc1[:], wT_sb[:, h * E + 128:(h + 1) * E], xT_h[:],
                        start=(h == 0), stop=(h == KT - 1),
                    )

                # scores^T [2x128e, 512t] -> SBUF
                scT0 = scT_pool.tile([128, 512], F32, tag="scT0")
                scT1 = scT_pool.tile([128, 512], F32, tag="scT1")
                nc.scalar.activation(scT0[:], ps_sc0[:], mybir.ActivationFunctionType.Copy)
                nc.vector.tensor_copy(scT1[:], ps_sc1[:])

                ov = outp.tile([128, 4 * OUTW], F32, tag="ov")
                for tt in range(4):
                    ps_s = pss_pool.tile([128, 256], F32, tag="pss")
                    nc.tensor.transpose(ps_s[:, 0:128], scT0[:, tt * 128:(tt + 1) * 128], ident[:])
                    nc.tensor.transpose(ps_s[:, 128:256], scT1[:, tt * 128:(tt + 1) * 128], ident[:])
                    # sigmoid fused into PSUM->SBUF move
                    sc = sc_pool.tile([128, E], F32, tag="sc")
                    nc.scalar.activation(sc[:], ps_s[:], mybir.ActivationFunctionType.Sigmoid)
                    scb = sc_pool.tile([128, E], F32, tag="scb")
                    nc.vector.tensor_add(scb[:], sc[:], biasb[:])

                    mv = ov[:, tt * OUTW:tt * OUTW + 8]
                    nc.vector.max(mv, scb[:])
                    nc.vector.max_index(ov[:, tt * OUTW + 8:tt * OUTW + 16].bitcast(U32), mv, scb[:])
                    s = small.tile([128, 1], F32, tag="s")
                    nc.vector.reduce_sum(s[:], mv, axis=mybir.AxisListType.X)
                    r = small.tile([128, 1], F32, tag="r")
                    nc.vector.reciprocal(r[:], s[:])
                    # w = mv * r * SCALE
                    nc.vector.tensor_scalar(
                        mv, mv, r[:], float(SCALE),
                        op0=mybir.AluOpType.mult, op1=mybir.AluOpType.mult,
                    )

                nc.sync.dma_start(
                    out[tg * 4:(tg + 1) * 4, :, :].rearrange("a p w -> p a w"),
                    ov[:].rearrange("p (a w) -> p a w", a=4),
                )

    nc.compile()
    return nc


_NC_CACHE = {}


def _get_nc(reps=1):
    if reps not in _NC_CACHE:
        _NC_CACHE[reps] = build_nc(reps)
    return _NC_CACHE[reps]


def kernel(x, weight, bias, _trace=False):
    x = np.asarray(x, dtype=np.float32)
    weight = np.asarray(weight, dtype=np.float32)
    bias = np.asarray(bias, dtype=np.float32)

    t = x.reshape(-1, H)                                   # [16384, 4096]
    wT = np.ascontiguousarray(weight.T)                    # [4096, 256]
    biasb = np.broadcast_to(bias, (128, E)).copy()         # [128, 256]

    nc = _get_nc()
    in_maps = [
        dict(x=np.ascontiguousarray(t[c * T_LOC:(c + 1) * T_LOC]), wT=wT, biasb=biasb)
        for c in range(N_CORES)
    ]
    res = run_bass_kernel_spmd(nc, in_maps, core_ids=list(range(N_CORES)), trace=_trace)

    outs = [r["out"].reshape(T_LOC, OUTW) for r in res.results]
    full = np.concatenate(outs, axis=0)                    # [16384, 16]
    topk_weight = np.ascontiguousarray(full[:, 0:8])
    topk_idx = np.ascontiguousarray(full[:, 8:16]).view(np.uint32).astype(np.int32)
    if _trace:
        return topk_idx, topk_weight, res
    return topk_idx, topk_weight
